# revision 16
# baseline (speedup 1.0000x reference)
"""Trainium2 Bass kernel for nn_Memory_efficient_network.

kernel(**inputs) takes the FULL unsharded inputs (as from setup_inputs())
and returns the full output tuple matching reference().

8-core strategy:
  shard = (batch b, trailing-2 k, half h); h halves C for gxc, P for pxg.
  L0 pools are host-precomputed; the 3x-channel concat is shipped
  pre-concatenated (48 ch) so L0 is one K=48 matmul per tile.
  L1 broadcast-pool terms are injected into PSUM by extra matmuls with
  stride-0 (broadcast) moving-operand APs.  lrelu+bias fused into ScalarE
  activation(Lrelu) at PSUM evacuation.  Pool partials are folded per-tile
  on VectorE, fully overlapped with L0.  Cross-core traffic: two pairwise
  AllGathers (g1 of gxc after gxc-L0, g2 of pxg after pxg-L0), each hidden
  behind subsequent compute.  Device emits only pooled_gxc / pooled_pxg;
  heads and tiny branches finish on host (fp32).
Device math: fp16 operands, fp32 PSUM accumulation.
"""

import numpy as np

NEG = 0.01
B, CH, NF, G, C, P = 2, 16, 64, 256, 512, 128
CL = C // 2
PL = P // 2
NCORES = 8
NQ_G = 64          # gxc banks (4 g each)
NQ_P = 16          # pxg banks (4 p each)

_cache = {}


# ===================================================================
# device program
# ===================================================================
def _build_runner():
    import jax
    from jax.sharding import Mesh, PartitionSpec
    from jax.experimental.shard_map import shard_map
    import concourse.bacc as bacc
    import concourse.mybir as mybir
    from concourse.tile import TileContext
    from concourse.bass2jax import (_bass_exec_p, install_neuronx_cc_hook,
                                    partition_id_tensor)

    dt = mybir.dt
    Alu = mybir.AluOpType
    Act = mybir.ActivationFunctionType

    nc = bacc.Bacc("TRN2", target_bir_lowering=False, debug=False,
                   num_devices=NCORES)

    xg_d = nc.dram_tensor("xg", [48, G * CL], dt.float16, kind="ExternalInput")
    xp_d = nc.dram_tensor("xp", [48, PL * G], dt.float16, kind="ExternalInput")
    wg0_d = nc.dram_tensor("wg0", [112, 64], dt.float16, kind="ExternalInput")
    wp0_d = nc.dram_tensor("wp0", [112, 64], dt.float16, kind="ExternalInput")
    w1_names = ["w1gx_x", "w1gx_g", "w1gx_c", "w1px_x", "w1px_p", "w1px_g"]
    w1_d = {n: nc.dram_tensor(n, [128, 64], dt.float16, kind="ExternalInput")
            for n in w1_names}
    bv_d = nc.dram_tensor("bv", [128, 4], dt.float32, kind="ExternalInput")
    pg_d = nc.dram_tensor("pg", [64, CL], dt.float32, kind="ExternalOutput")
    pp_d = nc.dram_tensor("pp", [128, NQ_P * 2], dt.float32,
                          kind="ExternalOutput")
    cc1_in = nc.dram_tensor("cc1_in", [128, 128], dt.float16)
    cc1_out = nc.dram_tensor("cc1_out", [256, 128], dt.float16)
    cc2_in = nc.dram_tensor("cc2_in", [128, 256], dt.float16)
    cc2_out = nc.dram_tensor("cc2_out", [256, 256], dt.float16)
    groups = [[0, 1], [2, 3], [4, 5], [6, 7]]

    with TileContext(nc) as tc:
        with tc.tile_pool(name="wpool", bufs=1) as wp, \
             tc.tile_pool(name="big", bufs=1) as bigp, \
             tc.tile_pool(name="chunk", bufs=3) as chp, \
             tc.tile_pool(name="evac", bufs=3) as evp, \
             tc.tile_pool(name="small", bufs=1) as smp, \
             tc.tile_pool(name="ps", bufs=2, space="PSUM") as psp:

            wg0 = wp.tile([112, 64], dt.float16, tag="wg0")
            wp0 = wp.tile([112, 64], dt.float16, tag="wp0")
            nc.sync.dma_start(wg0[:], wg0_d.ap())
            nc.sync.dma_start(wp0[:], wp0_d.ap())
            w1 = {}
            for n in w1_names:
                w1[n] = wp.tile([128, 64], dt.float16, tag=n, name=n)
                nc.sync.dma_start(w1[n][:], w1_d[n].ap())
            bv = wp.tile([128, 4], dt.float32, tag="bv")
            nc.sync.dma_start(bv[:], bv_d.ap())

            X1 = bigp.tile([128, NQ_G * 512], dt.float16, tag="X1")
            Y1 = bigp.tile([128, NQ_P * 512], dt.float16, tag="Y1")
            # pool partials (fold layout) + accumulators
            g1f = smp.tile([128, NQ_G * 2], dt.float16, tag="g1f")
            p1f = smp.tile([128, NQ_P * 2], dt.float16, tag="p1f")
            c1a = smp.tile([128, 256], dt.float16, tag="c1a")
            g2a = smp.tile([128, 256], dt.float16, tag="g2a")

            def fold_c(ev_ap, dst, slot):
                """inner fold (max over innermost 256) of a [128,2048] AP
                -> write [128, (4,2)] into dst[:, slot*8: slot*8+8]."""
                cur = ev_ap.rearrange("p (q g s) -> p q g s", q=4, g=2)
                w = 256
                while w > 1:
                    hw = w // 2
                    o = evp.tile([128, 4 * 2 * hw], dt.float16, tag="foldc",
                                 name=f"fc_{slot}_{hw}")
                    ov = o[:].rearrange("p (q g s) -> p q g s", q=4, g=2)
                    nc.vector.tensor_tensor(out=ov, in0=cur[:, :, :, 0:hw],
                                            in1=cur[:, :, :, hw:w],
                                            op=Alu.max)
                    cur, w = ov, hw
                nc.vector.tensor_copy(
                    dst[:, slot * 8:slot * 8 + 8]
                    .rearrange("p (q g) -> p q g", g=2).unsqueeze(3), cur)

            def fold_banks(ev_ap, acc, first):
                """max over the tile's 4 banks and glo -> acc [128,256]."""
                e4 = ev_ap.rearrange("p (a g s) -> p a g s", a=4, g=2)
                t1 = evp.tile([128, 1024], dt.float16, tag="pt1")
                t1v = t1[:].rearrange("p (a g s) -> p a g s", a=2, g=2)
                nc.vector.tensor_tensor(out=t1v, in0=e4[:, 0:2],
                                        in1=e4[:, 2:4], op=Alu.max)
                t2 = evp.tile([128, 512], dt.float16, tag="pt2")
                t2v = t2[:].rearrange("p (g s) -> p g s", g=2)
                nc.vector.tensor_tensor(out=t2v, in0=t1v[:, 0],
                                        in1=t1v[:, 1], op=Alu.max)
                if first:
                    nc.vector.tensor_tensor(out=acc[:], in0=t2v[:, 0],
                                            in1=t2v[:, 1], op=Alu.max)
                else:
                    t3 = evp.tile([128, 256], dt.float16, tag="pt3")
                    nc.vector.tensor_tensor(out=t3[:], in0=t2v[:, 0],
                                            in1=t2v[:, 1], op=Alu.max)
                    nc.vector.tensor_tensor(out=acc[:], in0=acc[:],
                                            in1=t3[:], op=Alu.max)

            # ---------------- layer 0 (fused pooling) ----------------
            def layer0(src_d, w0, Xbuf, nq, bias_col, innerf_dst, bank_acc):
                nchunk = nq // 8
                src = src_d.ap().rearrange("c (m u v s) -> c m v u s",
                                           m=nchunk, u=4, v=2)
                for m in range(nchunk):
                    t = chp.tile([112, 4096], dt.float16, tag="l0chunk")
                    tv = t[:].rearrange("p (u s) -> p u s", u=4)
                    nc.sync.dma_start(tv[0:48], src[:, m, 0])
                    nc.sync.dma_start(tv[64:112], src[:, m, 1])
                    for tt in range(2):
                        ps = psp.tile([128, 2048], dt.float32, tag="ps")
                        for qq in range(4):
                            j = 4 * tt + qq
                            u, v = j // 2, j % 2
                            R = 64 * v
                            off = qq * 512
                            for half in range(2):
                                nc.tensor.matmul(
                                    ps[64 * half:64 * half + 64,
                                       off:off + 512],
                                    w0[R:R + 48, :],
                                    t[R:R + 48, 1024 * u + 512 * half:
                                      1024 * u + 512 * half + 512],
                                    start=True, stop=True,
                                    tile_position=(R, 64 * half))
                        tile_idx = 2 * m + tt
                        ev = Xbuf[:, tile_idx * 2048:(tile_idx + 1) * 2048]
                        nc.scalar.activation(
                            ev, ps[:], Act.Lrelu,
                            bias=bv[:, bias_col:bias_col + 1], alpha=NEG)
                        # fused pools read the evacuated SBUF slice
                        fold_c(ev, innerf_dst, tile_idx)
                        fold_banks(ev, bank_acc, tile_idx == 0)

            layer0(xg_d, wg0, X1, NQ_G, 0, g1f, c1a)

            def half_combine(h128, tagbase):
                s = smp.tile([64, 256], dt.float16, tag=tagbase + "s",
                             name=tagbase + "s")
                nc.sync.dma_start(s[:], h128[64:128])
                d = smp.tile([128, 256], dt.float16, tag=tagbase + "d",
                             name=tagbase + "d")
                nc.vector.tensor_tensor(out=d[0:64], in0=h128[0:64],
                                        in1=s[:], op=Alu.max)
                nc.sync.dma_start(d[64:128], d[0:64])
                return d

            # c1 (local only) + exchange g1 (cc1)
            c1d = half_combine(c1a[:], "c1")
            nc.sync.dma_start(cc1_in.ap(), g1f[:])
            nc.gpsimd.collective_compute(
                "AllGather", Alu.bypass, replica_groups=groups,
                ins=[cc1_in.ap()], outs=[cc1_out.ap()])
            b10 = smp.tile([128, 128], dt.float16, tag="b10")
            b11 = smp.tile([128, 128], dt.float16, tag="b11")
            nc.sync.dma_start(b10[:], cc1_out.ap()[0:128])
            nc.sync.dma_start(b11[:], cc1_out.ap()[128:256])
            g1M = smp.tile([128, 128], dt.float16, tag="g1M")
            nc.vector.tensor_tensor(out=g1M[:], in0=b10[:], in1=b11[:],
                                    op=Alu.max)
            nc.vector.tensor_tensor(out=g1M[:], in0=g1M[:], in1=g1f[:],
                                    op=Alu.max)
            g1F = g1M[:].rearrange("p (q g) -> p q g", g=2)

            # pxg layer 0
            layer0(xp_d, wp0, Y1, NQ_P, 1, p1f, g2a)

            # g2 combine + exchange (cc2)
            g2d = half_combine(g2a[:], "g2")
            nc.sync.dma_start(cc2_in.ap(), g2d[:])
            nc.gpsimd.collective_compute(
                "AllGather", Alu.bypass, replica_groups=groups,
                ins=[cc2_in.ap()], outs=[cc2_out.ap()])
            b20 = smp.tile([128, 256], dt.float16, tag="b20")
            b21 = smp.tile([128, 256], dt.float16, tag="b21")
            nc.sync.dma_start(b20[:], cc2_out.ap()[0:128])
            nc.sync.dma_start(b21[:], cc2_out.ap()[128:256])
            g2M = smp.tile([128, 256], dt.float16, tag="g2M")
            nc.vector.tensor_tensor(out=g2M[:], in0=b20[:], in1=b21[:],
                                    op=Alu.max)
            nc.vector.tensor_tensor(out=g2M[:], in0=g2M[:], in1=g2d[:],
                                    op=Alu.max)
            g2F = g2M[:]
            p1F = p1f[:].rearrange("p (q g) -> p q g", g=2)

            # ---------------- layer 1 ----------------
            accg = smp.tile([128, 256], dt.float16, tag="accg")
            ppf = smp.tile([128, NQ_P * 2], dt.float32, tag="ppf")

            def l1_tile(Xbuf, nq, m, wx, wa, wb, a_rhs, b_rhs, bias_col):
                Xv = Xbuf[:].rearrange("p (q s) -> p q s", q=nq)
                ps = psp.tile([128, 2048], dt.float32, tag="ps",
                              name=f"psl1_{nq}_{m}")
                for qq in range(4):
                    q = 4 * m + qq
                    off = qq * 512
                    for half in range(2):
                        R = 64 * half
                        out = ps[R:R + 64, off:off + 512]
                        o3 = out.rearrange("p (g s) -> p g s", g=2)
                        # order: main -> B -> A (A depends on the exchange)
                        nc.tensor.matmul(out, wx[R:R + 64, :],
                                         Xv[R:R + 64, q],
                                         start=True, stop=False,
                                         tile_position=(R, R))
                        nc.tensor.matmul(o3, wb[R:R + 64, :], b_rhs(R, q),
                                         start=False, stop=False,
                                         tile_position=(R, R))
                        nc.tensor.matmul(o3, wa[R:R + 64, :], a_rhs(R, q),
                                         start=False, stop=True,
                                         tile_position=(R, R))
                ev = evp.tile([128, 2048], dt.float16, tag="evac",
                              name=f"ev1_{nq}_{m}")
                nc.scalar.activation(ev[:], ps[:], Act.Lrelu,
                                     bias=bv[:, bias_col:bias_col + 1],
                                     alpha=NEG)
                return ev

            def gxc_tile(m):
                ev = l1_tile(
                    X1, NQ_G, m, w1["w1gx_x"], w1["w1gx_g"], w1["w1gx_c"],
                    lambda R, q: g1F[R:R + 64, q].unsqueeze(2)
                                     .broadcast_to([64, 2, 256]),
                    lambda R, q: c1d[R:R + 64, :].unsqueeze(1)
                                     .broadcast_to([64, 2, 256]), 2)
                fold_banks(ev[:], accg, m == 0)

            def pxg_tile(m):
                ev = l1_tile(
                    Y1, NQ_P, m, w1["w1px_x"], w1["w1px_p"], w1["w1px_g"],
                    lambda R, q: p1F[R:R + 64, q].unsqueeze(2)
                                     .broadcast_to([64, 2, 256]),
                    lambda R, q: g2F[R:R + 64, :].unsqueeze(1)
                                     .broadcast_to([64, 2, 256]), 3)
                # pooled_pxg: inner fold over g -> [128, 8] into ppf
                cur = ev[:].rearrange("p (q g s) -> p q g s", q=4, g=2)
                w = 256
                while w > 1:
                    hw = w // 2
                    o = evp.tile([128, 4 * 2 * hw], dt.float16, tag="foldc",
                                 name=f"pxf_{m}_{hw}")
                    ov = o[:].rearrange("p (q g s) -> p q g s", q=4, g=2)
                    nc.vector.tensor_tensor(out=ov, in0=cur[:, :, :, 0:hw],
                                            in1=cur[:, :, :, hw:w],
                                            op=Alu.max)
                    cur, w = ov, hw
                nc.vector.tensor_copy(
                    ppf[:, m * 8:(m + 1) * 8]
                    .rearrange("p (q g) -> p q g", g=2).unsqueeze(3), cur)

            for m in range(12):
                gxc_tile(m)
            rest = [(gxc_tile, 12), (pxg_tile, 0), (gxc_tile, 13),
                    (pxg_tile, 1), (gxc_tile, 14), (pxg_tile, 2),
                    (gxc_tile, 15), (pxg_tile, 3)]
            for fn_, m in rest:
                fn_(m)

            # pooled_gxc out: cross-half combine -> [64,256] fp32
            pgs = smp.tile([64, 256], dt.float16, tag="pgs")
            nc.sync.dma_start(pgs[:], accg[64:128])
            pgf = smp.tile([64, 256], dt.float32, tag="pgf")
            nc.vector.tensor_tensor(out=pgf[:], in0=accg[0:64], in1=pgs[:],
                                    op=Alu.max)
            nc.sync.dma_start(pg_d.ap(), pgf[:])
            nc.sync.dma_start(pp_d.ap(), ppf[:])

    nc.compile()
    nc.finalize()

    # ------------- hoisted-jit runner -------------
    install_neuronx_cc_hook()
    import concourse.mybir as mybir_m
    partition_name = (nc.partition_id_tensor.name
                      if nc.partition_id_tensor else None)
    in_names, out_names, out_avals, zero_outs = [], [], [], []
    for alloc in nc.m.functions[0].allocations:
        if not isinstance(alloc, mybir_m.MemoryLocationSet):
            continue
        name = alloc.memorylocations[0].name
        if alloc.kind == "ExternalInput":
            if name != partition_name:
                in_names.append(name)
        elif alloc.kind == "ExternalOutput":
            out_names.append(name)
            shape = tuple(alloc.tensor_shape)
            dtp = mybir_m.dt.np(alloc.dtype)
            out_avals.append(jax.core.ShapedArray(shape, dtp))
            zero_outs.append(np.zeros(shape, dtp))
    n_params, n_outs = len(in_names), len(out_avals)
    all_in_names = in_names + out_names + (
        [partition_name] if partition_name else [])

    def _body(*args):
        operands = list(args)
        if partition_name:
            operands.append(partition_id_tensor())
        outs = _bass_exec_p.bind(
            *operands, out_avals=tuple(out_avals),
            in_names=tuple(all_in_names), out_names=tuple(out_names),
            lowering_input_output_aliases=(), sim_require_finite=True,
            sim_require_nnan=True, nc=nc)
        return tuple(outs)

    devices = jax.devices()[:NCORES]
    mesh = Mesh(np.asarray(devices), ("core",))
    in_specs = (PartitionSpec("core"),) * (n_params + n_outs)
    out_specs = (PartitionSpec("core"),) * n_outs
    fn = jax.jit(shard_map(_body, mesh=mesh, in_specs=in_specs,
                           out_specs=out_specs, check_rep=False),
                 keep_unused=True)

    def run(in_maps):
        concat_in = [np.concatenate([in_maps[c][nm] for c in range(NCORES)],
                                    axis=0) for nm in in_names]
        concat_zeros = [np.zeros((NCORES * z.shape[0], *z.shape[1:]), z.dtype)
                        for z in zero_outs]
        out_arrs = fn(*concat_in, *concat_zeros)
        return [
            {name: np.asarray(out_arrs[i]).reshape(NCORES,
                                                   *out_avals[i].shape)[c]
             for i, name in enumerate(out_names)}
            for c in range(NCORES)]

    _cache.update(dict(nc=nc, in_names=in_names, out_names=out_names,
                       out_avals=out_avals, zero_outs=zero_outs,
                       partition_name=partition_name, mesh=mesh,
                       jax=jax))
    return run


# ===================================================================
# host side
# ===================================================================
def _lrelu(x):
    return np.maximum(x, NEG * x)


def _pconv(x, w, b):
    return (np.einsum('oc,bc...->bo...', w, x, optimize=True)
            + b.reshape((1, -1) + (1,) * (x.ndim - 2)))


def _prep_inputs(input_GxCx2, input_PxGx2, params):
    f16 = np.float16
    p = params
    w0g = np.ascontiguousarray(np.asarray(p['w_gxc0']).T).astype(f16)
    w0p = np.ascontiguousarray(np.asarray(p['w_pxg0']).T).astype(f16)
    wg0 = np.zeros((112, 64), f16); wg0[0:48] = w0g; wg0[64:112] = w0g
    wp0 = np.zeros((112, 64), f16); wp0[0:48] = w0p; wp0[64:112] = w0p

    def dupT(w):
        t = np.ascontiguousarray(np.asarray(w).T).astype(f16)
        return np.concatenate([t, t], axis=0)
    w1 = {
        "w1gx_x": dupT(p['w_gxc1'][:, 0:64]),
        "w1gx_g": dupT(p['w_gxc1'][:, 64:128]),
        "w1gx_c": dupT(p['w_gxc1'][:, 128:192]),
        "w1px_x": dupT(p['w_pxg1'][:, 0:64]),
        "w1px_p": dupT(p['w_pxg1'][:, 64:128]),
        "w1px_g": dupT(p['w_pxg1'][:, 128:192]),
    }
    bvv = np.zeros((128, 4), np.float32)
    for i, n in enumerate(['b_gxc0', 'b_pxg0', 'b_gxc1', 'b_pxg1']):
        bvv[0:64, i] = p[n]; bvv[64:128, i] = p[n]

    def make_aug(x, pool_a, pool_b):
        ch, O, I = x.shape
        aug = np.empty((48, O, I), np.float32)
        aug[0:16] = x
        aug[16:32] = pool_a[:, :, None]
        aug[32:48] = pool_b[:, None, :]
        return np.ascontiguousarray(aug).astype(f16).reshape(48, O * I)

    maps = []
    xg_f = np.asarray(input_GxCx2, np.float32)
    xp_f = np.asarray(input_PxGx2, np.float32)
    for b in range(B):
        for k in range(2):
            xg = xg_f[b, :, :, :, k]
            g1_0 = xg.max(axis=2)
            c1_0 = xg.max(axis=1)
            xp = xp_f[b, :, :, :, k]
            p1_0 = xp.max(axis=2)
            g2_0 = xp.max(axis=1)
            for h in range(2):
                m = {
                    "xg": make_aug(xg[:, :, h * CL:(h + 1) * CL], g1_0,
                                   c1_0[:, h * CL:(h + 1) * CL]),
                    "xp": make_aug(xp[:, h * PL:(h + 1) * PL, :],
                                   p1_0[:, h * PL:(h + 1) * PL], g2_0),
                    "wg0": wg0, "wp0": wp0, "bv": bvv,
                }
                m.update(w1)
                maps.append(m)
    return maps


def _decode_pool_layout(vec128, nq):
    v = vec128.reshape(2, 64, nq, 2)
    out = np.empty((64, nq * 4), vec128.dtype)
    idx = np.arange(nq)[:, None] * 4 + np.arange(2)[None, :]
    out[:, idx.ravel()] = v[0].reshape(64, nq * 2)
    out[:, (idx + 2).ravel()] = v[1].reshape(64, nq * 2)
    return out


def kernel(input_GxCx2, input_PxGx2, input_P, input_G, input_1, params):
    if "run" not in _cache:
        _cache["run"] = _build_runner()
    run = _cache["run"]

    maps = _prep_inputs(input_GxCx2, input_PxGx2, params)
    results = run(maps)

    pooled_gxc = np.empty((B, 64, C, 2), np.float32)
    pooled_pxg = np.empty((B, 64, P, 2), np.float32)
    ci = 0
    for b in range(B):
        for k in range(2):
            for h in range(2):
                r = results[ci]; ci += 1
                pooled_gxc[b, :, h * CL:(h + 1) * CL, k] = r["pg"]
                pooled_pxg[b, :, h * PL:(h + 1) * PL, k] = \
                    _decode_pool_layout(r["pp"], NQ_P)

    p = {k2: np.asarray(v, np.float32) for k2, v in params.items()}
    pp_ = np.asarray(input_P, np.float32)
    gg = np.asarray(input_G, np.float32)
    one = None
    for i in range(2):
        pp_ = _lrelu(_pconv(pp_, p[f'w_p{i}'], p[f'b_p{i}']))
        gg = _lrelu(_pconv(gg, p[f'w_g{i}'], p[f'b_g{i}']))
        one = _lrelu(_pconv(gg, p[f'w_1{i}'], p[f'b_1{i}']))

    def bc4(a, b_):
        a2 = np.broadcast_to(a.max(2, keepdims=True), a.shape)
        b2 = np.broadcast_to(b_.max(2, keepdims=True), b_.shape)
        return (np.concatenate([a, a2], 1), np.concatenate([b_, b2], 1))

    a_gxc, a_pxg = bc4(pooled_gxc, pooled_pxg)
    out_a_gxc = _pconv(a_gxc, p['w_act_gxc'], p['b_act_gxc'])
    out_a_pxg = _pconv(a_pxg, p['w_act_pxg'], p['b_act_pxg'])
    out_a_p = _pconv(pp_, p['w_act_p'], p['b_act_p'])
    out_a_g = _pconv(gg, p['w_act_g'], p['b_act_g'])
    out_a_1 = _pconv(one, p['w_act_1'], p['b_act_1'])
    v_gxc, v_pxg = bc4(pooled_gxc, pooled_pxg)
    v1 = _pconv(v_gxc, p['w_cri_gxc'], p['b_cri_gxc'])
    v2 = _pconv(v_pxg, p['w_cri_pxg'], p['b_cri_pxg'])
    v3 = _pconv(pp_, p['w_cri_p'], p['b_cri_p'])
    v4 = _pconv(gg, p['w_cri_g'], p['b_cri_g'])
    v5 = _pconv(one, p['w_cri_1'], p['b_cri_1'])
    value = np.array([v1.mean(3).sum() + v2.mean(3).sum() + v3.sum()
                      + v4.sum() + v5.sum()], np.float32)
    return (out_a_gxc.astype(np.float32), out_a_pxg.astype(np.float32),
            out_a_p.astype(np.float32), out_a_g.astype(np.float32),
            out_a_1.astype(np.float32), value)


# revision 20
# speedup vs baseline: 1.3588x; 1.3588x over previous
"""Trainium2 Bass kernel for nn_Memory_efficient_network.

kernel(**inputs) takes the FULL unsharded inputs (as from setup_inputs())
and returns the full output tuple matching reference().

8-core strategy:
  shard = (batch b, trailing-2 k, half h); h halves C for gxc, P for pxg.
  L0 pools are host-precomputed; the 3x-channel concat is shipped
  pre-concatenated (48 ch) so L0 is one K=48 matmul per tile.
  L1 broadcast-pool terms are injected into PSUM by extra matmuls with
  stride-0 (broadcast) moving-operand APs.  lrelu+bias fused into ScalarE
  activation(Lrelu) at PSUM evacuation.  Pool partials are folded per-tile
  on VectorE, fully overlapped with L0.  Cross-core traffic: two pairwise
  AllGathers (g1 of gxc after gxc-L0, g2 of pxg after pxg-L0), each hidden
  behind subsequent compute.  Device emits only pooled_gxc / pooled_pxg;
  heads and tiny branches finish on host (fp32).
Device math: fp16 operands, fp32 PSUM accumulation.
"""

import numpy as np

NEG = 0.01
B, CH, NF, G, C, P = 2, 16, 64, 256, 512, 128
CL = C // 2
PL = P // 2
NCORES = 8
NQ_G = 64          # gxc banks (4 g each)
NQ_P = 16          # pxg banks (4 p each)

_cache = {}


# ===================================================================
# device program
# ===================================================================
def _build_runner():
    import jax
    from jax.sharding import Mesh, PartitionSpec
    from jax.experimental.shard_map import shard_map
    import concourse.bacc as bacc
    import concourse.mybir as mybir
    from concourse.tile import TileContext
    from concourse.bass2jax import (_bass_exec_p, install_neuronx_cc_hook,
                                    partition_id_tensor)

    dt = mybir.dt
    Alu = mybir.AluOpType
    Act = mybir.ActivationFunctionType

    nc = bacc.Bacc("TRN2", target_bir_lowering=False, debug=False,
                   num_devices=NCORES)

    xg_d = nc.dram_tensor("xg", [48, G * CL], dt.float16, kind="ExternalInput")
    xp_d = nc.dram_tensor("xp", [48, PL * G], dt.float16, kind="ExternalInput")
    wc_d = nc.dram_tensor("wc", [128, 512], dt.float16, kind="ExternalInput")
    w1_names = ["w1gx_x", "w1gx_g", "w1gx_c", "w1px_x", "w1px_p", "w1px_g"]
    bv_d = nc.dram_tensor("bv", [128, 4], dt.float32, kind="ExternalInput")
    pg_d = nc.dram_tensor("pg", [64, CL], dt.float32, kind="ExternalOutput")
    pp_d = nc.dram_tensor("pp", [128, NQ_P * 2], dt.float32,
                          kind="ExternalOutput")
    cc1_in = nc.dram_tensor("cc1_in", [128, 128], dt.float16)
    cc1_out = nc.dram_tensor("cc1_out", [256, 128], dt.float16)
    cc2_in = nc.dram_tensor("cc2_in", [128, 256], dt.float16)
    cc2_out = nc.dram_tensor("cc2_out", [256, 256], dt.float16)
    groups = [[0, 1], [2, 3], [4, 5], [6, 7]]

    with TileContext(nc) as tc:
        with tc.tile_pool(name="wpool", bufs=1) as wp, \
             tc.tile_pool(name="big", bufs=1) as bigp, \
             tc.tile_pool(name="chunk", bufs=3) as chp, \
             tc.tile_pool(name="evac", bufs=3) as evp, \
             tc.tile_pool(name="small", bufs=1) as smp, \
             tc.tile_pool(name="ps", bufs=2, space="PSUM") as psp:

            wc = wp.tile([128, 512], dt.float16, tag="wc")
            nc.sync.dma_start(wc[:], wc_d.ap())
            wg0 = wc[:, 0:64]
            wp0 = wc[:, 64:128]
            w1 = {n: wc[:, 128 + i * 64:192 + i * 64]
                  for i, n in enumerate(w1_names)}
            bv = wp.tile([128, 4], dt.float32, tag="bv")
            nc.sync.dma_start(bv[:], bv_d.ap())

            X1 = bigp.tile([128, NQ_G * 512], dt.float16, tag="X1")
            Y1 = bigp.tile([128, NQ_P * 512], dt.float16, tag="Y1")
            # pool partials (fold layout) + accumulators
            g1f = smp.tile([128, NQ_G * 2], dt.float16, tag="g1f")
            p1f = smp.tile([128, NQ_P * 2], dt.float16, tag="p1f")
            c1a = smp.tile([128, 256], dt.float16, tag="c1a")
            g2a = smp.tile([128, 256], dt.float16, tag="g2a")

            def big_inner_fold(src_ap, nb, dst, off):
                """max over innermost 256 of [128, nb*512] -> dst[:, off:off+nb*2]"""
                cur = src_ap.rearrange("p (q g s) -> p q g s", q=nb, g=2)
                w = 256
                while w > 1:
                    hw = w // 2
                    o = evp.tile([128, nb * 2 * hw], dt.float16, tag="trA",
                                 name=f"bif_{off}_{nb}_{hw}", bufs=2)
                    ov = o[:].rearrange("p (q g s) -> p q g s", q=nb, g=2)
                    nc.vector.tensor_tensor(out=ov, in0=cur[:, :, :, 0:hw],
                                            in1=cur[:, :, :, hw:w],
                                            op=Alu.max)
                    cur, w = ov, hw
                nc.vector.tensor_copy(
                    dst[:, off:off + nb * 2]
                    .rearrange("p (q g) -> p q g", g=2).unsqueeze(3), cur)

            def big_bank_tree(src_ap, nb, acc, first):
                """max over banks+glo of [128, nb*512] -> acc [128,256]"""
                cur = src_ap
                n = nb
                while n > 1:
                    v = cur.rearrange("p (a two s) -> p a two s", two=2, s=512)
                    o = evp.tile([128, (n // 2) * 512], dt.float16, tag="trA",
                                 name=f"bbt_{nb}_{n}_{first}", bufs=2)
                    ov = o[:].rearrange("p (a s) -> p a s", s=512)
                    nc.vector.tensor_tensor(out=ov, in0=v[:, :, 0],
                                            in1=v[:, :, 1], op=Alu.max)
                    cur, n = o[:], n // 2
                v = cur.rearrange("p (g s) -> p g s", g=2)
                if first:
                    nc.vector.tensor_tensor(out=acc[:], in0=v[:, 0],
                                            in1=v[:, 1], op=Alu.max)
                else:
                    o = evp.tile([128, 256], dt.float16, tag="trB",
                                 name=f"bbt2_{nb}_{first}")
                    nc.vector.tensor_tensor(out=o[:], in0=v[:, 0],
                                            in1=v[:, 1], op=Alu.max)
                    nc.vector.tensor_tensor(out=acc[:], in0=acc[:],
                                            in1=o[:], op=Alu.max)

            # ---------------- layer 0 ----------------
            def layer0(src_d, w0, Xbuf, nq, bias_col, half_hook):
                nchunk = nq // 8
                src = src_d.ap().rearrange("c (m u v s) -> c m v u s",
                                           m=nchunk, u=4, v=2)
                for m in range(nchunk):
                    t = chp.tile([112, 4096], dt.float16, tag="l0chunk")
                    tv = t[:].rearrange("p (u s) -> p u s", u=4)
                    nc.sync.dma_start(tv[0:48], src[:, m, 0])
                    nc.sync.dma_start(tv[64:112], src[:, m, 1])
                    for tt in range(2):
                        ps = psp.tile([128, 2048], dt.float32, tag="ps")
                        for qq in range(4):
                            j = 4 * tt + qq
                            u, v = j // 2, j % 2
                            R = 64 * v
                            off = qq * 512
                            for half in range(2):
                                nc.tensor.matmul(
                                    ps[64 * half:64 * half + 64,
                                       off:off + 512],
                                    w0[R:R + 48, :],
                                    t[R:R + 48, 1024 * u + 512 * half:
                                      1024 * u + 512 * half + 512],
                                    start=True, stop=True,
                                    tile_position=(R, 64 * half))
                        tile_idx = 2 * m + tt
                        ev = Xbuf[:, tile_idx * 2048:(tile_idx + 1) * 2048]
                        nc.scalar.activation(
                            ev, ps[:], Act.Lrelu,
                            bias=bv[:, bias_col:bias_col + 1], alpha=NEG)
                        if half_hook is not None and tile_idx == nq // 8 - 1:
                            half_hook()

            # gxc layer 0 with trees on first half overlapped
            def gxc_half1_trees():
                big_inner_fold(X1[:, 0:NQ_G * 256], NQ_G // 2, g1f, 0)
                big_bank_tree(X1[:, 0:NQ_G * 256], NQ_G // 2, c1a, True)

            layer0(xg_d, wg0, X1, NQ_G, 0, gxc_half1_trees)
            big_inner_fold(X1[:, NQ_G * 256:NQ_G * 512], NQ_G // 2, g1f,
                           NQ_G)
            # g1 ready -> exchange (cc1) immediately
            nc.sync.dma_start(cc1_in.ap(), g1f[:])
            nc.gpsimd.collective_compute(
                "AllGather", Alu.bypass, replica_groups=groups,
                ins=[cc1_in.ap()], outs=[cc1_out.ap()])
            big_bank_tree(X1[:, NQ_G * 256:NQ_G * 512], NQ_G // 2, c1a,
                          False)

            def half_combine(h128, tagbase):
                s = smp.tile([64, 256], dt.float16, tag=tagbase + "s",
                             name=tagbase + "s")
                nc.sync.dma_start(s[:], h128[64:128])
                d = smp.tile([128, 256], dt.float16, tag=tagbase + "d",
                             name=tagbase + "d")
                nc.vector.tensor_tensor(out=d[0:64], in0=h128[0:64],
                                        in1=s[:], op=Alu.max)
                nc.sync.dma_start(d[64:128], d[0:64])
                return d

            c1d = half_combine(c1a[:], "c1")

            # pxg layer 0 + its trees + cc2
            layer0(xp_d, wp0, Y1, NQ_P, 1, None)
            big_inner_fold(Y1[:], NQ_P, p1f, 0)
            big_bank_tree(Y1[:], NQ_P, g2a, True)
            g2d = half_combine(g2a[:], "g2")
            nc.sync.dma_start(cc2_in.ap(), g2d[:])
            nc.gpsimd.collective_compute(
                "AllGather", Alu.bypass, replica_groups=groups,
                ins=[cc2_in.ap()], outs=[cc2_out.ap()])

            # cc1 merge
            b10 = smp.tile([128, 128], dt.float16, tag="b10")
            b11 = smp.tile([128, 128], dt.float16, tag="b11")
            nc.sync.dma_start(b10[:], cc1_out.ap()[0:128])
            nc.sync.dma_start(b11[:], cc1_out.ap()[128:256])
            g1M = smp.tile([128, 128], dt.float16, tag="g1M")
            nc.vector.tensor_tensor(out=g1M[:], in0=b10[:], in1=b11[:],
                                    op=Alu.max)
            nc.vector.tensor_tensor(out=g1M[:], in0=g1M[:], in1=g1f[:],
                                    op=Alu.max)
            g1F = g1M[:].rearrange("p (q g) -> p q g", g=2)

            # cc2 merge
            b20 = smp.tile([128, 256], dt.float16, tag="b20")
            b21 = smp.tile([128, 256], dt.float16, tag="b21")
            nc.sync.dma_start(b20[:], cc2_out.ap()[0:128])
            nc.sync.dma_start(b21[:], cc2_out.ap()[128:256])
            g2M = smp.tile([128, 256], dt.float16, tag="g2M")
            nc.vector.tensor_tensor(out=g2M[:], in0=b20[:], in1=b21[:],
                                    op=Alu.max)
            nc.vector.tensor_tensor(out=g2M[:], in0=g2M[:], in1=g2d[:],
                                    op=Alu.max)
            g2F = g2M[:]
            p1F = p1f[:].rearrange("p (q g) -> p q g", g=2)

            # ---------------- layer 1 ----------------
            acc2048 = smp.tile([128, 2048], dt.float16, tag="acc2048")
            ppf = smp.tile([128, NQ_P * 2], dt.float32, tag="ppf")

            def l1_tile(Xbuf, nq, m, wx, wa, wb, a_rhs, b_rhs, bias_col):
                Xv = Xbuf[:].rearrange("p (q s) -> p q s", q=nq)
                ps = psp.tile([128, 2048], dt.float32, tag="ps",
                              name=f"psl1_{nq}_{m}")
                regions = [(qq, half) for qq in range(4) for half in range(2)]
                for qq, half in regions:
                    R = 64 * half
                    out = ps[R:R + 64, qq * 512:qq * 512 + 512]
                    nc.tensor.matmul(out, wx[R:R + 64, :],
                                     Xv[R:R + 64, 4 * m + qq],
                                     start=True, stop=False,
                                     tile_position=(R, R))
                for qq, half in regions:
                    R = 64 * half
                    o3 = ps[R:R + 64, qq * 512:qq * 512 + 512].rearrange(
                        "p (g s) -> p g s", g=2)
                    nc.tensor.matmul(o3, wb[R:R + 64, :],
                                     b_rhs(R, 4 * m + qq),
                                     start=False, stop=False,
                                     tile_position=(R, R))
                for qq, half in regions:
                    R = 64 * half
                    o3 = ps[R:R + 64, qq * 512:qq * 512 + 512].rearrange(
                        "p (g s) -> p g s", g=2)
                    nc.tensor.matmul(o3, wa[R:R + 64, :],
                                     a_rhs(R, 4 * m + qq),
                                     start=False, stop=True,
                                     tile_position=(R, R))
                ev = evp.tile([128, 2048], dt.float16, tag="evac",
                              name=f"ev1_{nq}_{m}")
                nc.scalar.activation(ev[:], ps[:], Act.Lrelu,
                                     bias=bv[:, bias_col:bias_col + 1],
                                     alpha=NEG)
                return ev

            def gxc_tile(m):
                ev = l1_tile(
                    X1, NQ_G, m, w1["w1gx_x"], w1["w1gx_g"], w1["w1gx_c"],
                    lambda R, q: g1F[R:R + 64, q].unsqueeze(2)
                                     .broadcast_to([64, 2, 256]),
                    lambda R, q: c1d[R:R + 64, :].unsqueeze(1)
                                     .broadcast_to([64, 2, 256]), 2)
                if m == 0:
                    nc.vector.tensor_copy(acc2048[:], ev[:])
                else:
                    nc.vector.tensor_tensor(out=acc2048[:], in0=acc2048[:],
                                            in1=ev[:], op=Alu.max)

            def pxg_tile(m):
                ev = l1_tile(
                    Y1, NQ_P, m, w1["w1px_x"], w1["w1px_p"], w1["w1px_g"],
                    lambda R, q: p1F[R:R + 64, q].unsqueeze(2)
                                     .broadcast_to([64, 2, 256]),
                    lambda R, q: g2F[R:R + 64, :].unsqueeze(1)
                                     .broadcast_to([64, 2, 256]), 3)
                cur = ev[:].rearrange("p (q g s) -> p q g s", q=4, g=2)
                for hw in (128, 64):
                    o = evp.tile([128, 4 * 2 * hw], dt.float16, tag="foldc",
                                 name=f"pxf_{m}_{hw}")
                    ov = o[:].rearrange("p (q g s) -> p q g s", q=4, g=2)
                    nc.vector.tensor_tensor(out=ov, in0=cur[:, :, :, 0:hw],
                                            in1=cur[:, :, :, hw:2 * hw],
                                            op=Alu.max)
                    cur = ov
                red = evp.tile([128, 8], dt.float16, tag="pxred",
                               name=f"pxr_{m}")
                nc.vector.tensor_reduce(
                    red[:].rearrange("p (q g) -> p q g", g=2).unsqueeze(3),
                    cur, axis=mybir.AxisListType.X, op=Alu.max)
                nc.vector.tensor_copy(
                    ppf[:, m * 8:(m + 1) * 8]
                    .rearrange("p (q g) -> p q g", g=2).unsqueeze(3),
                    red[:].rearrange("p (q g) -> p q g", g=2).unsqueeze(3))

            for m in range(8):
                gxc_tile(m)
            for m in range(4):
                pxg_tile(m)
            for m in range(8, 16):
                gxc_tile(m)

            # final gxc pool: fold acc2048 -> accg [128,256]
            accg = smp.tile([128, 256], dt.float16, tag="accg")
            fb = acc2048[:].rearrange("p (a g s) -> p a g s", a=4, g=2)
            tq1 = evp.tile([128, 1024], dt.float16, tag="pt1")
            tq1v = tq1[:].rearrange("p (a g s) -> p a g s", a=2, g=2)
            nc.vector.tensor_tensor(out=tq1v, in0=fb[:, 0:2], in1=fb[:, 2:4],
                                    op=Alu.max)
            tq2 = evp.tile([128, 512], dt.float16, tag="pt2")
            tq2v = tq2[:].rearrange("p (g s) -> p g s", g=2)
            nc.vector.tensor_tensor(out=tq2v, in0=tq1v[:, 0], in1=tq1v[:, 1],
                                    op=Alu.max)
            nc.vector.tensor_tensor(out=accg[:], in0=tq2v[:, 0],
                                    in1=tq2v[:, 1], op=Alu.max)

            # pooled_gxc out: cross-half combine -> [64,256] fp32
            pgs = smp.tile([64, 256], dt.float16, tag="pgs")
            nc.sync.dma_start(pgs[:], accg[64:128])
            pgf = smp.tile([64, 256], dt.float32, tag="pgf")
            nc.vector.tensor_tensor(out=pgf[:], in0=accg[0:64], in1=pgs[:],
                                    op=Alu.max)
            nc.sync.dma_start(pg_d.ap(), pgf[:])
            nc.sync.dma_start(pp_d.ap(), ppf[:])

    nc.compile()
    nc.finalize()

    # ------------- hoisted-jit runner -------------
    install_neuronx_cc_hook()
    import concourse.mybir as mybir_m
    partition_name = (nc.partition_id_tensor.name
                      if nc.partition_id_tensor else None)
    in_names, out_names, out_avals, zero_outs = [], [], [], []
    for alloc in nc.m.functions[0].allocations:
        if not isinstance(alloc, mybir_m.MemoryLocationSet):
            continue
        name = alloc.memorylocations[0].name
        if alloc.kind == "ExternalInput":
            if name != partition_name:
                in_names.append(name)
        elif alloc.kind == "ExternalOutput":
            out_names.append(name)
            shape = tuple(alloc.tensor_shape)
            dtp = mybir_m.dt.np(alloc.dtype)
            out_avals.append(jax.core.ShapedArray(shape, dtp))
            zero_outs.append(np.zeros(shape, dtp))
    n_params, n_outs = len(in_names), len(out_avals)
    all_in_names = in_names + out_names + (
        [partition_name] if partition_name else [])

    def _body(*args):
        operands = list(args)
        if partition_name:
            operands.append(partition_id_tensor())
        outs = _bass_exec_p.bind(
            *operands, out_avals=tuple(out_avals),
            in_names=tuple(all_in_names), out_names=tuple(out_names),
            lowering_input_output_aliases=(), sim_require_finite=True,
            sim_require_nnan=True, nc=nc)
        return tuple(outs)

    devices = jax.devices()[:NCORES]
    mesh = Mesh(np.asarray(devices), ("core",))
    in_specs = (PartitionSpec("core"),) * (n_params + n_outs)
    out_specs = (PartitionSpec("core"),) * n_outs
    fn = jax.jit(shard_map(_body, mesh=mesh, in_specs=in_specs,
                           out_specs=out_specs, check_rep=False),
                 keep_unused=True)

    def run(in_maps):
        concat_in = [np.concatenate([in_maps[c][nm] for c in range(NCORES)],
                                    axis=0) for nm in in_names]
        concat_zeros = [np.zeros((NCORES * z.shape[0], *z.shape[1:]), z.dtype)
                        for z in zero_outs]
        out_arrs = fn(*concat_in, *concat_zeros)
        return [
            {name: np.asarray(out_arrs[i]).reshape(NCORES,
                                                   *out_avals[i].shape)[c]
             for i, name in enumerate(out_names)}
            for c in range(NCORES)]

    _cache.update(dict(nc=nc, in_names=in_names, out_names=out_names,
                       out_avals=out_avals, zero_outs=zero_outs,
                       partition_name=partition_name, mesh=mesh,
                       jax=jax))
    return run


# ===================================================================
# host side
# ===================================================================
def _lrelu(x):
    return np.maximum(x, NEG * x)


def _pconv(x, w, b):
    return (np.einsum('oc,bc...->bo...', w, x, optimize=True)
            + b.reshape((1, -1) + (1,) * (x.ndim - 2)))


def _prep_inputs(input_GxCx2, input_PxGx2, params):
    f16 = np.float16
    p = params
    w0g = np.ascontiguousarray(np.asarray(p['w_gxc0']).T).astype(f16)
    w0p = np.ascontiguousarray(np.asarray(p['w_pxg0']).T).astype(f16)
    wg0 = np.zeros((128, 64), f16); wg0[0:48] = w0g; wg0[64:112] = w0g
    wp0 = np.zeros((128, 64), f16); wp0[0:48] = w0p; wp0[64:112] = w0p

    def dupT(w):
        t = np.ascontiguousarray(np.asarray(w).T).astype(f16)
        return np.concatenate([t, t], axis=0)
    w1_names = ["w1gx_x", "w1gx_g", "w1gx_c", "w1px_x", "w1px_p", "w1px_g"]
    w1 = {
        "w1gx_x": dupT(p['w_gxc1'][:, 0:64]),
        "w1gx_g": dupT(p['w_gxc1'][:, 64:128]),
        "w1gx_c": dupT(p['w_gxc1'][:, 128:192]),
        "w1px_x": dupT(p['w_pxg1'][:, 0:64]),
        "w1px_p": dupT(p['w_pxg1'][:, 64:128]),
        "w1px_g": dupT(p['w_pxg1'][:, 128:192]),
    }
    wcomb = np.zeros((128, 512), f16)
    wcomb[:, 0:64] = wg0
    wcomb[:, 64:128] = wp0
    for i, n in enumerate(w1_names):
        wcomb[:, 128 + i * 64:192 + i * 64] = w1[n]
    bvv = np.zeros((128, 4), np.float32)
    for i, n in enumerate(['b_gxc0', 'b_pxg0', 'b_gxc1', 'b_pxg1']):
        bvv[0:64, i] = p[n]; bvv[64:128, i] = p[n]

    def make_aug(x, pool_a, pool_b):
        ch, O, I = x.shape
        aug = np.empty((48, O, I), np.float32)
        aug[0:16] = x
        aug[16:32] = pool_a[:, :, None]
        aug[32:48] = pool_b[:, None, :]
        return np.ascontiguousarray(aug).astype(f16).reshape(48, O * I)

    maps = []
    xg_f = np.asarray(input_GxCx2, np.float32)
    xp_f = np.asarray(input_PxGx2, np.float32)
    for b in range(B):
        for k in range(2):
            xg = xg_f[b, :, :, :, k]
            g1_0 = xg.max(axis=2)
            c1_0 = xg.max(axis=1)
            xp = xp_f[b, :, :, :, k]
            p1_0 = xp.max(axis=2)
            g2_0 = xp.max(axis=1)
            for h in range(2):
                m = {
                    "xg": make_aug(xg[:, :, h * CL:(h + 1) * CL], g1_0,
                                   c1_0[:, h * CL:(h + 1) * CL]),
                    "xp": make_aug(xp[:, h * PL:(h + 1) * PL, :],
                                   p1_0[:, h * PL:(h + 1) * PL], g2_0),
                    "wc": wcomb, "bv": bvv,
                }
                maps.append(m)
    return maps


def _decode_pool_layout(vec128, nq):
    v = vec128.reshape(2, 64, nq, 2)
    out = np.empty((64, nq * 4), vec128.dtype)
    idx = np.arange(nq)[:, None] * 4 + np.arange(2)[None, :]
    out[:, idx.ravel()] = v[0].reshape(64, nq * 2)
    out[:, (idx + 2).ravel()] = v[1].reshape(64, nq * 2)
    return out


def kernel(input_GxCx2, input_PxGx2, input_P, input_G, input_1, params):
    if "run" not in _cache:
        _cache["run"] = _build_runner()
    run = _cache["run"]

    maps = _prep_inputs(input_GxCx2, input_PxGx2, params)
    results = run(maps)

    pooled_gxc = np.empty((B, 64, C, 2), np.float32)
    pooled_pxg = np.empty((B, 64, P, 2), np.float32)
    ci = 0
    for b in range(B):
        for k in range(2):
            for h in range(2):
                r = results[ci]; ci += 1
                pooled_gxc[b, :, h * CL:(h + 1) * CL, k] = r["pg"]
                pooled_pxg[b, :, h * PL:(h + 1) * PL, k] = \
                    _decode_pool_layout(r["pp"], NQ_P)

    p = {k2: np.asarray(v, np.float32) for k2, v in params.items()}
    pp_ = np.asarray(input_P, np.float32)
    gg = np.asarray(input_G, np.float32)
    one = None
    for i in range(2):
        pp_ = _lrelu(_pconv(pp_, p[f'w_p{i}'], p[f'b_p{i}']))
        gg = _lrelu(_pconv(gg, p[f'w_g{i}'], p[f'b_g{i}']))
        one = _lrelu(_pconv(gg, p[f'w_1{i}'], p[f'b_1{i}']))

    def bc4(a, b_):
        a2 = np.broadcast_to(a.max(2, keepdims=True), a.shape)
        b2 = np.broadcast_to(b_.max(2, keepdims=True), b_.shape)
        return (np.concatenate([a, a2], 1), np.concatenate([b_, b2], 1))

    a_gxc, a_pxg = bc4(pooled_gxc, pooled_pxg)
    out_a_gxc = _pconv(a_gxc, p['w_act_gxc'], p['b_act_gxc'])
    out_a_pxg = _pconv(a_pxg, p['w_act_pxg'], p['b_act_pxg'])
    out_a_p = _pconv(pp_, p['w_act_p'], p['b_act_p'])
    out_a_g = _pconv(gg, p['w_act_g'], p['b_act_g'])
    out_a_1 = _pconv(one, p['w_act_1'], p['b_act_1'])
    v_gxc, v_pxg = bc4(pooled_gxc, pooled_pxg)
    v1 = _pconv(v_gxc, p['w_cri_gxc'], p['b_cri_gxc'])
    v2 = _pconv(v_pxg, p['w_cri_pxg'], p['b_cri_pxg'])
    v3 = _pconv(pp_, p['w_cri_p'], p['b_cri_p'])
    v4 = _pconv(gg, p['w_cri_g'], p['b_cri_g'])
    v5 = _pconv(one, p['w_cri_1'], p['b_cri_1'])
    value = np.array([v1.mean(3).sum() + v2.mean(3).sum() + v3.sum()
                      + v4.sum() + v5.sum()], np.float32)
    return (out_a_gxc.astype(np.float32), out_a_pxg.astype(np.float32),
            out_a_p.astype(np.float32), out_a_g.astype(np.float32),
            out_a_1.astype(np.float32), value)


# revision 21
# speedup vs baseline: 5829.3602x; 4290.1879x over previous
"""Trainium2 Bass kernel for nn_Memory_efficient_network.

kernel(**inputs) takes the FULL unsharded inputs (as from setup_inputs())
and returns the full output tuple matching reference().

8-core strategy:
  shard = (batch b, trailing-2 k, half h); h halves C for gxc, P for pxg.
  L0 pools are host-precomputed; the 3x-channel concat is shipped
  pre-concatenated (48 ch) so L0 is one K=48 matmul per tile.
  L1 broadcast-pool terms are injected into PSUM by extra matmuls with
  stride-0 (broadcast) moving-operand APs.  lrelu+bias fused into ScalarE
  activation(Lrelu) at PSUM evacuation.  Pool partials are folded per-tile
  on VectorE, fully overlapped with L0.  Cross-core traffic: two pairwise
  AllGathers (g1 of gxc after gxc-L0, g2 of pxg after pxg-L0), each hidden
  behind subsequent compute.  Device emits only pooled_gxc / pooled_pxg;
  heads and tiny branches finish on host (fp32).
Device math: fp16 operands, fp32 PSUM accumulation.
"""

import numpy as np

NEG = 0.01
B, CH, NF, G, C, P = 2, 16, 64, 256, 512, 128
CL = C // 2
PL = P // 2
NCORES = 8
NQ_G = 64          # gxc banks (4 g each)
NQ_P = 16          # pxg banks (4 p each)

_cache = {}


# ===================================================================
# device program
# ===================================================================
def _build_runner():
    import jax
    from jax.sharding import Mesh, PartitionSpec
    from jax.experimental.shard_map import shard_map
    import concourse.bacc as bacc
    import concourse.mybir as mybir
    from concourse.tile import TileContext
    from concourse.bass2jax import (_bass_exec_p, install_neuronx_cc_hook,
                                    partition_id_tensor)

    dt = mybir.dt
    Alu = mybir.AluOpType
    Act = mybir.ActivationFunctionType

    nc = bacc.Bacc("TRN2", target_bir_lowering=False, debug=False,
                   num_devices=NCORES)

    xg_d = nc.dram_tensor("xg", [48, G * CL], dt.float16, kind="ExternalInput")
    xp_d = nc.dram_tensor("xp", [48, PL * G], dt.float16, kind="ExternalInput")
    wc_d = nc.dram_tensor("wc", [128, 512], dt.float16, kind="ExternalInput")
    w1_names = ["w1gx_x", "w1gx_g", "w1gx_c", "w1px_x", "w1px_p", "w1px_g"]
    bv_d = nc.dram_tensor("bv", [128, 4], dt.float32, kind="ExternalInput")
    pg_d = nc.dram_tensor("pg", [64, CL], dt.float32, kind="ExternalOutput")
    pp_d = nc.dram_tensor("pp", [128, NQ_P * 2], dt.float32,
                          kind="ExternalOutput")
    cc1_in = nc.dram_tensor("cc1_in", [128, 128], dt.float16)
    cc1_out = nc.dram_tensor("cc1_out", [256, 128], dt.float16)
    cc2_in = nc.dram_tensor("cc2_in", [128, 256], dt.float16)
    cc2_out = nc.dram_tensor("cc2_out", [256, 256], dt.float16)
    groups = [[0, 1], [2, 3], [4, 5], [6, 7]]

    with TileContext(nc) as tc:
        with tc.tile_pool(name="wpool", bufs=1) as wp, \
             tc.tile_pool(name="big", bufs=1) as bigp, \
             tc.tile_pool(name="chunk", bufs=3) as chp, \
             tc.tile_pool(name="evac", bufs=3) as evp, \
             tc.tile_pool(name="small", bufs=1) as smp, \
             tc.tile_pool(name="ps", bufs=2, space="PSUM") as psp:

            wc = wp.tile([128, 512], dt.float16, tag="wc")
            nc.sync.dma_start(wc[:], wc_d.ap())
            wg0 = wc[:, 0:64]
            wp0 = wc[:, 64:128]
            w1 = {n: wc[:, 128 + i * 64:192 + i * 64]
                  for i, n in enumerate(w1_names)}
            bv = wp.tile([128, 4], dt.float32, tag="bv")
            nc.sync.dma_start(bv[:], bv_d.ap())

            X1 = bigp.tile([128, NQ_G * 512], dt.float16, tag="X1")
            Y1 = bigp.tile([128, NQ_P * 512], dt.float16, tag="Y1")
            # pool partials (fold layout) + accumulators
            g1f = smp.tile([128, NQ_G * 2], dt.float16, tag="g1f")
            p1f = smp.tile([128, NQ_P * 2], dt.float16, tag="p1f")
            c1a = smp.tile([128, 256], dt.float16, tag="c1a")
            g2a = smp.tile([128, 256], dt.float16, tag="g2a")

            def big_inner_fold(src_ap, nb, dst, off):
                """max over innermost 256 of [128, nb*512] -> dst[:, off:off+nb*2]"""
                cur = src_ap.rearrange("p (q g s) -> p q g s", q=nb, g=2)
                w = 256
                while w > 1:
                    hw = w // 2
                    o = evp.tile([128, nb * 2 * hw], dt.float16, tag="trA",
                                 name=f"bif_{off}_{nb}_{hw}", bufs=2)
                    ov = o[:].rearrange("p (q g s) -> p q g s", q=nb, g=2)
                    nc.vector.tensor_tensor(out=ov, in0=cur[:, :, :, 0:hw],
                                            in1=cur[:, :, :, hw:w],
                                            op=Alu.max)
                    cur, w = ov, hw
                nc.vector.tensor_copy(
                    dst[:, off:off + nb * 2]
                    .rearrange("p (q g) -> p q g", g=2).unsqueeze(3), cur)

            def big_bank_tree(src_ap, nb, acc, first):
                """max over banks+glo of [128, nb*512] -> acc [128,256]"""
                cur = src_ap
                n = nb
                while n > 1:
                    v = cur.rearrange("p (a two s) -> p a two s", two=2, s=512)
                    o = evp.tile([128, (n // 2) * 512], dt.float16, tag="trA",
                                 name=f"bbt_{nb}_{n}_{first}", bufs=2)
                    ov = o[:].rearrange("p (a s) -> p a s", s=512)
                    nc.vector.tensor_tensor(out=ov, in0=v[:, :, 0],
                                            in1=v[:, :, 1], op=Alu.max)
                    cur, n = o[:], n // 2
                v = cur.rearrange("p (g s) -> p g s", g=2)
                if first:
                    nc.vector.tensor_tensor(out=acc[:], in0=v[:, 0],
                                            in1=v[:, 1], op=Alu.max)
                else:
                    o = evp.tile([128, 256], dt.float16, tag="trB",
                                 name=f"bbt2_{nb}_{first}")
                    nc.vector.tensor_tensor(out=o[:], in0=v[:, 0],
                                            in1=v[:, 1], op=Alu.max)
                    nc.vector.tensor_tensor(out=acc[:], in0=acc[:],
                                            in1=o[:], op=Alu.max)

            # ---------------- layer 0 ----------------
            def layer0(src_d, w0, Xbuf, nq, bias_col, half_hook):
                nchunk = nq // 8
                src = src_d.ap().rearrange("c (m u v s) -> c m v u s",
                                           m=nchunk, u=4, v=2)
                for m in range(nchunk):
                    t = chp.tile([112, 4096], dt.float16, tag="l0chunk")
                    tv = t[:].rearrange("p (u s) -> p u s", u=4)
                    nc.sync.dma_start(tv[0:48], src[:, m, 0])
                    nc.sync.dma_start(tv[64:112], src[:, m, 1])
                    for tt in range(2):
                        ps = psp.tile([128, 2048], dt.float32, tag="ps")
                        for qq in range(4):
                            j = 4 * tt + qq
                            u, v = j // 2, j % 2
                            R = 64 * v
                            off = qq * 512
                            for half in range(2):
                                nc.tensor.matmul(
                                    ps[64 * half:64 * half + 64,
                                       off:off + 512],
                                    w0[R:R + 48, :],
                                    t[R:R + 48, 1024 * u + 512 * half:
                                      1024 * u + 512 * half + 512],
                                    start=True, stop=True,
                                    tile_position=(R, 64 * half))
                        tile_idx = 2 * m + tt
                        ev = Xbuf[:, tile_idx * 2048:(tile_idx + 1) * 2048]
                        nc.scalar.activation(
                            ev, ps[:], Act.Lrelu,
                            bias=bv[:, bias_col:bias_col + 1], alpha=NEG)
                        if half_hook is not None and tile_idx == nq // 8 - 1:
                            half_hook()

            # gxc layer 0 with trees on first half overlapped
            def gxc_half1_trees():
                big_inner_fold(X1[:, 0:NQ_G * 256], NQ_G // 2, g1f, 0)
                big_bank_tree(X1[:, 0:NQ_G * 256], NQ_G // 2, c1a, True)

            layer0(xg_d, wg0, X1, NQ_G, 0, gxc_half1_trees)
            big_inner_fold(X1[:, NQ_G * 256:NQ_G * 512], NQ_G // 2, g1f,
                           NQ_G)
            # g1 ready -> exchange (cc1) immediately
            nc.sync.dma_start(cc1_in.ap(), g1f[:])
            nc.gpsimd.collective_compute(
                "AllGather", Alu.bypass, replica_groups=groups,
                ins=[cc1_in.ap()], outs=[cc1_out.ap()])
            big_bank_tree(X1[:, NQ_G * 256:NQ_G * 512], NQ_G // 2, c1a,
                          False)

            def half_combine(h128, tagbase):
                s = smp.tile([64, 256], dt.float16, tag=tagbase + "s",
                             name=tagbase + "s")
                nc.sync.dma_start(s[:], h128[64:128])
                d = smp.tile([128, 256], dt.float16, tag=tagbase + "d",
                             name=tagbase + "d")
                nc.vector.tensor_tensor(out=d[0:64], in0=h128[0:64],
                                        in1=s[:], op=Alu.max)
                nc.sync.dma_start(d[64:128], d[0:64])
                return d

            c1d = half_combine(c1a[:], "c1")

            # pxg layer 0 + its trees + cc2
            layer0(xp_d, wp0, Y1, NQ_P, 1, None)
            # cc1 merge
            b10 = smp.tile([128, 128], dt.float16, tag="b10")
            b11 = smp.tile([128, 128], dt.float16, tag="b11")
            nc.sync.dma_start(b10[:], cc1_out.ap()[0:128])
            nc.sync.dma_start(b11[:], cc1_out.ap()[128:256])
            g1M = smp.tile([128, 128], dt.float16, tag="g1M")
            nc.vector.tensor_tensor(out=g1M[:], in0=b10[:], in1=b11[:],
                                    op=Alu.max)
            nc.vector.tensor_tensor(out=g1M[:], in0=g1M[:], in1=g1f[:],
                                    op=Alu.max)
            g1F = g1M[:].rearrange("p (q g) -> p q g", g=2)

            big_inner_fold(Y1[:], NQ_P, p1f, 0)
            big_bank_tree(Y1[:], NQ_P, g2a, True)
            g2d = half_combine(g2a[:], "g2")
            nc.sync.dma_start(cc2_in.ap(), g2d[:])
            nc.gpsimd.collective_compute(
                "AllGather", Alu.bypass, replica_groups=groups,
                ins=[cc2_in.ap()], outs=[cc2_out.ap()])

            # cc2 merge
            b20 = smp.tile([128, 256], dt.float16, tag="b20")
            b21 = smp.tile([128, 256], dt.float16, tag="b21")
            nc.sync.dma_start(b20[:], cc2_out.ap()[0:128])
            nc.sync.dma_start(b21[:], cc2_out.ap()[128:256])
            g2M = smp.tile([128, 256], dt.float16, tag="g2M")
            nc.vector.tensor_tensor(out=g2M[:], in0=b20[:], in1=b21[:],
                                    op=Alu.max)
            nc.vector.tensor_tensor(out=g2M[:], in0=g2M[:], in1=g2d[:],
                                    op=Alu.max)
            g2F = g2M[:]
            p1F = p1f[:].rearrange("p (q g) -> p q g", g=2)

            # ---------------- layer 1 ----------------
            acc2048 = smp.tile([128, 2048], dt.float16, tag="acc2048")
            ppf = smp.tile([128, NQ_P * 2], dt.float32, tag="ppf")

            def l1_tile(Xbuf, nq, m, wx, wa, wb, a_rhs, b_rhs, bias_col):
                Xv = Xbuf[:].rearrange("p (q s) -> p q s", q=nq)
                ps = psp.tile([128, 2048], dt.float32, tag="ps",
                              name=f"psl1_{nq}_{m}")
                regions = [(qq, half) for qq in range(4) for half in range(2)]
                for qq, half in regions:
                    R = 64 * half
                    out = ps[R:R + 64, qq * 512:qq * 512 + 512]
                    nc.tensor.matmul(out, wx[R:R + 64, :],
                                     Xv[R:R + 64, 4 * m + qq],
                                     start=True, stop=False,
                                     tile_position=(R, R))
                for qq, half in regions:
                    R = 64 * half
                    o3 = ps[R:R + 64, qq * 512:qq * 512 + 512].rearrange(
                        "p (g s) -> p g s", g=2)
                    nc.tensor.matmul(o3, wb[R:R + 64, :],
                                     b_rhs(R, 4 * m + qq),
                                     start=False, stop=False,
                                     tile_position=(R, R))
                for qq, half in regions:
                    R = 64 * half
                    o3 = ps[R:R + 64, qq * 512:qq * 512 + 512].rearrange(
                        "p (g s) -> p g s", g=2)
                    nc.tensor.matmul(o3, wa[R:R + 64, :],
                                     a_rhs(R, 4 * m + qq),
                                     start=False, stop=True,
                                     tile_position=(R, R))
                ev = evp.tile([128, 2048], dt.float16, tag="evac",
                              name=f"ev1_{nq}_{m}")
                nc.scalar.activation(ev[:], ps[:], Act.Lrelu,
                                     bias=bv[:, bias_col:bias_col + 1],
                                     alpha=NEG)
                return ev

            def gxc_tile(m):
                ev = l1_tile(
                    X1, NQ_G, m, w1["w1gx_x"], w1["w1gx_g"], w1["w1gx_c"],
                    lambda R, q: g1F[R:R + 64, q].unsqueeze(2)
                                     .broadcast_to([64, 2, 256]),
                    lambda R, q: c1d[R:R + 64, :].unsqueeze(1)
                                     .broadcast_to([64, 2, 256]), 2)
                if m == 0:
                    nc.vector.tensor_copy(acc2048[:], ev[:])
                else:
                    nc.vector.tensor_tensor(out=acc2048[:], in0=acc2048[:],
                                            in1=ev[:], op=Alu.max)

            def pxg_tile(m):
                ev = l1_tile(
                    Y1, NQ_P, m, w1["w1px_x"], w1["w1px_p"], w1["w1px_g"],
                    lambda R, q: p1F[R:R + 64, q].unsqueeze(2)
                                     .broadcast_to([64, 2, 256]),
                    lambda R, q: g2F[R:R + 64, :].unsqueeze(1)
                                     .broadcast_to([64, 2, 256]), 3)
                cur = ev[:].rearrange("p (q g s) -> p q g s", q=4, g=2)
                for hw in (128, 64):
                    o = evp.tile([128, 4 * 2 * hw], dt.float16, tag="foldc",
                                 name=f"pxf_{m}_{hw}")
                    ov = o[:].rearrange("p (q g s) -> p q g s", q=4, g=2)
                    nc.vector.tensor_tensor(out=ov, in0=cur[:, :, :, 0:hw],
                                            in1=cur[:, :, :, hw:2 * hw],
                                            op=Alu.max)
                    cur = ov
                red = evp.tile([128, 8], dt.float16, tag="pxred",
                               name=f"pxr_{m}")
                nc.vector.tensor_reduce(
                    red[:].rearrange("p (q g) -> p q g", g=2).unsqueeze(3),
                    cur, axis=mybir.AxisListType.X, op=Alu.max)
                nc.vector.tensor_copy(
                    ppf[:, m * 8:(m + 1) * 8]
                    .rearrange("p (q g) -> p q g", g=2).unsqueeze(3),
                    red[:].rearrange("p (q g) -> p q g", g=2).unsqueeze(3))

            for m in range(8):
                gxc_tile(m)
            for m in range(4):
                pxg_tile(m)
            for m in range(8, 16):
                gxc_tile(m)

            # final gxc pool: fold acc2048 -> accg [128,256]
            accg = smp.tile([128, 256], dt.float16, tag="accg")
            fb = acc2048[:].rearrange("p (a g s) -> p a g s", a=4, g=2)
            tq1 = evp.tile([128, 1024], dt.float16, tag="pt1")
            tq1v = tq1[:].rearrange("p (a g s) -> p a g s", a=2, g=2)
            nc.vector.tensor_tensor(out=tq1v, in0=fb[:, 0:2], in1=fb[:, 2:4],
                                    op=Alu.max)
            tq2 = evp.tile([128, 512], dt.float16, tag="pt2")
            tq2v = tq2[:].rearrange("p (g s) -> p g s", g=2)
            nc.vector.tensor_tensor(out=tq2v, in0=tq1v[:, 0], in1=tq1v[:, 1],
                                    op=Alu.max)
            nc.vector.tensor_tensor(out=accg[:], in0=tq2v[:, 0],
                                    in1=tq2v[:, 1], op=Alu.max)

            # pooled_gxc out: cross-half combine -> [64,256] fp32
            pgs = smp.tile([64, 256], dt.float16, tag="pgs")
            nc.sync.dma_start(pgs[:], accg[64:128])
            pgf = smp.tile([64, 256], dt.float32, tag="pgf")
            nc.vector.tensor_tensor(out=pgf[:], in0=accg[0:64], in1=pgs[:],
                                    op=Alu.max)
            nc.sync.dma_start(pg_d.ap(), pgf[:])
            nc.sync.dma_start(pp_d.ap(), ppf[:])

    nc.compile()
    nc.finalize()

    # ------------- hoisted-jit runner -------------
    install_neuronx_cc_hook()
    import concourse.mybir as mybir_m
    partition_name = (nc.partition_id_tensor.name
                      if nc.partition_id_tensor else None)
    in_names, out_names, out_avals, zero_outs = [], [], [], []
    for alloc in nc.m.functions[0].allocations:
        if not isinstance(alloc, mybir_m.MemoryLocationSet):
            continue
        name = alloc.memorylocations[0].name
        if alloc.kind == "ExternalInput":
            if name != partition_name:
                in_names.append(name)
        elif alloc.kind == "ExternalOutput":
            out_names.append(name)
            shape = tuple(alloc.tensor_shape)
            dtp = mybir_m.dt.np(alloc.dtype)
            out_avals.append(jax.core.ShapedArray(shape, dtp))
            zero_outs.append(np.zeros(shape, dtp))
    n_params, n_outs = len(in_names), len(out_avals)
    all_in_names = in_names + out_names + (
        [partition_name] if partition_name else [])

    def _body(*args):
        operands = list(args)
        if partition_name:
            operands.append(partition_id_tensor())
        outs = _bass_exec_p.bind(
            *operands, out_avals=tuple(out_avals),
            in_names=tuple(all_in_names), out_names=tuple(out_names),
            lowering_input_output_aliases=(), sim_require_finite=True,
            sim_require_nnan=True, nc=nc)
        return tuple(outs)

    devices = jax.devices()[:NCORES]
    mesh = Mesh(np.asarray(devices), ("core",))
    in_specs = (PartitionSpec("core"),) * (n_params + n_outs)
    out_specs = (PartitionSpec("core"),) * n_outs
    fn = jax.jit(shard_map(_body, mesh=mesh, in_specs=in_specs,
                           out_specs=out_specs, check_rep=False),
                 keep_unused=True)

    def run(in_maps):
        concat_in = [np.concatenate([in_maps[c][nm] for c in range(NCORES)],
                                    axis=0) for nm in in_names]
        concat_zeros = [np.zeros((NCORES * z.shape[0], *z.shape[1:]), z.dtype)
                        for z in zero_outs]
        out_arrs = fn(*concat_in, *concat_zeros)
        return [
            {name: np.asarray(out_arrs[i]).reshape(NCORES,
                                                   *out_avals[i].shape)[c]
             for i, name in enumerate(out_names)}
            for c in range(NCORES)]

    _cache.update(dict(nc=nc, in_names=in_names, out_names=out_names,
                       out_avals=out_avals, zero_outs=zero_outs,
                       partition_name=partition_name, mesh=mesh,
                       jax=jax))
    return run


# ===================================================================
# host side
# ===================================================================
def _lrelu(x):
    return np.maximum(x, NEG * x)


def _pconv(x, w, b):
    return (np.einsum('oc,bc...->bo...', w, x, optimize=True)
            + b.reshape((1, -1) + (1,) * (x.ndim - 2)))


def _prep_inputs(input_GxCx2, input_PxGx2, params):
    f16 = np.float16
    p = params
    w0g = np.ascontiguousarray(np.asarray(p['w_gxc0']).T).astype(f16)
    w0p = np.ascontiguousarray(np.asarray(p['w_pxg0']).T).astype(f16)
    wg0 = np.zeros((128, 64), f16); wg0[0:48] = w0g; wg0[64:112] = w0g
    wp0 = np.zeros((128, 64), f16); wp0[0:48] = w0p; wp0[64:112] = w0p

    def dupT(w):
        t = np.ascontiguousarray(np.asarray(w).T).astype(f16)
        return np.concatenate([t, t], axis=0)
    w1_names = ["w1gx_x", "w1gx_g", "w1gx_c", "w1px_x", "w1px_p", "w1px_g"]
    w1 = {
        "w1gx_x": dupT(p['w_gxc1'][:, 0:64]),
        "w1gx_g": dupT(p['w_gxc1'][:, 64:128]),
        "w1gx_c": dupT(p['w_gxc1'][:, 128:192]),
        "w1px_x": dupT(p['w_pxg1'][:, 0:64]),
        "w1px_p": dupT(p['w_pxg1'][:, 64:128]),
        "w1px_g": dupT(p['w_pxg1'][:, 128:192]),
    }
    wcomb = np.zeros((128, 512), f16)
    wcomb[:, 0:64] = wg0
    wcomb[:, 64:128] = wp0
    for i, n in enumerate(w1_names):
        wcomb[:, 128 + i * 64:192 + i * 64] = w1[n]
    bvv = np.zeros((128, 4), np.float32)
    for i, n in enumerate(['b_gxc0', 'b_pxg0', 'b_gxc1', 'b_pxg1']):
        bvv[0:64, i] = p[n]; bvv[64:128, i] = p[n]

    def make_aug(x, pool_a, pool_b):
        ch, O, I = x.shape
        aug = np.empty((48, O, I), np.float32)
        aug[0:16] = x
        aug[16:32] = pool_a[:, :, None]
        aug[32:48] = pool_b[:, None, :]
        return np.ascontiguousarray(aug).astype(f16).reshape(48, O * I)

    maps = []
    xg_f = np.asarray(input_GxCx2, np.float32)
    xp_f = np.asarray(input_PxGx2, np.float32)
    for b in range(B):
        for k in range(2):
            xg = xg_f[b, :, :, :, k]
            g1_0 = xg.max(axis=2)
            c1_0 = xg.max(axis=1)
            xp = xp_f[b, :, :, :, k]
            p1_0 = xp.max(axis=2)
            g2_0 = xp.max(axis=1)
            for h in range(2):
                m = {
                    "xg": make_aug(xg[:, :, h * CL:(h + 1) * CL], g1_0,
                                   c1_0[:, h * CL:(h + 1) * CL]),
                    "xp": make_aug(xp[:, h * PL:(h + 1) * PL, :],
                                   p1_0[:, h * PL:(h + 1) * PL], g2_0),
                    "wc": wcomb, "bv": bvv,
                }
                maps.append(m)
    return maps


def _decode_pool_layout(vec128, nq):
    v = vec128.reshape(2, 64, nq, 2)
    out = np.empty((64, nq * 4), vec128.dtype)
    idx = np.arange(nq)[:, None] * 4 + np.arange(2)[None, :]
    out[:, idx.ravel()] = v[0].reshape(64, nq * 2)
    out[:, (idx + 2).ravel()] = v[1].reshape(64, nq * 2)
    return out


def kernel(input_GxCx2, input_PxGx2, input_P, input_G, input_1, params):
    if "run" not in _cache:
        _cache["run"] = _build_runner()
    run = _cache["run"]

    maps = _prep_inputs(input_GxCx2, input_PxGx2, params)
    results = run(maps)

    pooled_gxc = np.empty((B, 64, C, 2), np.float32)
    pooled_pxg = np.empty((B, 64, P, 2), np.float32)
    ci = 0
    for b in range(B):
        for k in range(2):
            for h in range(2):
                r = results[ci]; ci += 1
                pooled_gxc[b, :, h * CL:(h + 1) * CL, k] = r["pg"]
                pooled_pxg[b, :, h * PL:(h + 1) * PL, k] = \
                    _decode_pool_layout(r["pp"], NQ_P)

    p = {k2: np.asarray(v, np.float32) for k2, v in params.items()}
    pp_ = np.asarray(input_P, np.float32)
    gg = np.asarray(input_G, np.float32)
    one = None
    for i in range(2):
        pp_ = _lrelu(_pconv(pp_, p[f'w_p{i}'], p[f'b_p{i}']))
        gg = _lrelu(_pconv(gg, p[f'w_g{i}'], p[f'b_g{i}']))
        one = _lrelu(_pconv(gg, p[f'w_1{i}'], p[f'b_1{i}']))

    def bc4(a, b_):
        a2 = np.broadcast_to(a.max(2, keepdims=True), a.shape)
        b2 = np.broadcast_to(b_.max(2, keepdims=True), b_.shape)
        return (np.concatenate([a, a2], 1), np.concatenate([b_, b2], 1))

    a_gxc, a_pxg = bc4(pooled_gxc, pooled_pxg)
    out_a_gxc = _pconv(a_gxc, p['w_act_gxc'], p['b_act_gxc'])
    out_a_pxg = _pconv(a_pxg, p['w_act_pxg'], p['b_act_pxg'])
    out_a_p = _pconv(pp_, p['w_act_p'], p['b_act_p'])
    out_a_g = _pconv(gg, p['w_act_g'], p['b_act_g'])
    out_a_1 = _pconv(one, p['w_act_1'], p['b_act_1'])
    v_gxc, v_pxg = bc4(pooled_gxc, pooled_pxg)
    v1 = _pconv(v_gxc, p['w_cri_gxc'], p['b_cri_gxc'])
    v2 = _pconv(v_pxg, p['w_cri_pxg'], p['b_cri_pxg'])
    v3 = _pconv(pp_, p['w_cri_p'], p['b_cri_p'])
    v4 = _pconv(gg, p['w_cri_g'], p['b_cri_g'])
    v5 = _pconv(one, p['w_cri_1'], p['b_cri_1'])
    value = np.array([v1.mean(3).sum() + v2.mean(3).sum() + v3.sum()
                      + v4.sum() + v5.sum()], np.float32)
    return (out_a_gxc.astype(np.float32), out_a_pxg.astype(np.float32),
            out_a_p.astype(np.float32), out_a_g.astype(np.float32),
            out_a_1.astype(np.float32), value)
